# revision 5
# baseline (speedup 1.0000x reference)
"""Single-head causal attention on 8 TRN2 NeuronCores — v2.

out[b,t,:] = softmax_causal((x Wq^T)(x Wk^T)^T / sqrt(C)) @ (x Wv^T)

Sharding: core = (batch b=core//2, parity p=core%2). Core owns q 512-blocks
{p, p+2, p+4, p+6}; uniform program positions i=0..3 run EMAIN=(4,12,20,28)
main key chunks + 4 diag chunks. Even-parity cores get 4 zero-padded key
chunks prepended; pads contribute nothing because their v-natural rows AND
indicator column are zero (no P masking needed).

Numerics: weights pre-scaled by 32 host-side so q,k,v land in fp8 e4m3
range. Projections and PV run fp8 DoubleRow (2 contract chunks/cycle-pair);
QK is fp8 (contract=64). exp is exact on ACT (scale folds the 32*32), with
fp8 P output feeding DoubleRow PV; an indicator column (=32) in v-natural
yields the softmax denominator through the same PV matmuls. Rows 0..127
(first diag chunk of position 0) use a bf16 P and bf16 v-natural path to
bound fp8 quantization error where few keys dominate.
"""

import math
import sys

for _p in ("/opt/trn_rl_repo",):
    if _p not in sys.path:
        sys.path.insert(0, _p)

import numpy as np
import ml_dtypes

BF16 = ml_dtypes.bfloat16
F8 = ml_dtypes.float8_e4m3

B, T, C, H = 4, 4096, 1024, 64
NCORES = 8
WS = 32.0                     # host-side weight scale
ESC = (C ** -0.5) / (WS * WS)  # exp scale on raw psum scores
EMAIN = (4, 12, 20, 28)       # uniform main-phase extents (128-key chunks)
# main pairs offloaded to DVE+GPSIMD fast-exp per position (large-neff rows)
_FEXP_GS = True
_FEXP_LAG = 4
_PSCB, _POPB, _PMB = 3, 1, 1
FEXP = ((), (2, 4), (2, 4, 6), (2, 5, 8, 11))
FE_A = float((2 ** 23) * ESC / np.log(2.0))   # Schraudolph scale
FE_B = float(1065353216.0 - 486411.0)         # bias, mean-centered
TOFF = (0, 512, 896, 1152)    # tight stair offsets in diag score/P tiles

def flush(pend, pv_diag, pv_main, stop):
    it = pend.pop(0)
    if it[0] == "dA":
        pv_diag(0)
    elif it[0] == "dB":
        pv_diag(1)
    else:
        pv_main(it[2], it[1], stop)


_CACHE = {}


def _build_program():
    import concourse.bass as bass
    import concourse.mybir as mybir
    import concourse.tile as tile
    from concourse import bacc
    from concourse.masks import make_identity

    f32 = mybir.dt.float32
    bf16 = mybir.dt.bfloat16
    f8 = mybir.dt.float8e4
    DR = mybir.MatmulPerfMode.DoubleRow
    Exp = mybir.ActivationFunctionType.Exp

    nc = bacc.Bacc("TRN2", target_bir_lowering=False, debug=False)
    xkv_d = [nc.dram_tensor(f"xkv{g}", [128, 8, 1024], f8,
                            kind="ExternalInput") for g in range(1, 4)]
    xkv0_d = [nc.dram_tensor(f"xkv0{h}", [128, 8, 512], f8,
                             kind="ExternalInput") for h in range(2)]
    wt_d = nc.dram_tensor("wt", [128, 8, 192], f8, kind="ExternalInput")
    # small blob per partition: [w0 3072B bf16][x0 2048B bf16][ind 16B f32]
    sb_d = nc.dram_tensor("smallblob", [128, 5136], mybir.dt.uint8,
                          kind="ExternalInput")
    out_d = nc.dram_tensor("out", [4 * 512, H], f32, kind="ExternalOutput")

    with tile.TileContext(nc) as tc:
        with tc.tile_pool(name="persist", bufs=1) as P, \
             tc.tile_pool(name="pscore", bufs=_PSCB, space="PSUM") as PSC, \
             tc.tile_pool(name="pop", bufs=_POPB, space="PSUM") as POP, \
             tc.tile_pool(name="pmisc", bufs=_PMB, space="PSUM") as PM, \
             tc.tile_pool(name="ppool", bufs=3) as PP, \
             tc.tile_pool(name="ppoolf", bufs=7) as PPF, \
             tc.tile_pool(name="fin", bufs=2) as FIN:
            xkv_sb = P.tile([128, 8, 4096], f8)
            wt_sb = P.tile([128, 8, 192], f8)
            sb_sb = P.tile([128, 5136], mybir.dt.uint8)
            w0_sb = sb_sb[:, 0:3072].bitcast(bf16) \
                .rearrange("p (c n) -> p c n", c=8)
            x0_sb = sb_sb[:, 3072:5120].bitcast(bf16) \
                .rearrange("p (c n) -> p c n", c=8)
            ind_sb = sb_sb[:, 5120:5136].bitcast(f32)
            q0b_sb = P.tile([64, 128], bf16)     # bf16 q rows 0..127 (pos 0)
            k0b_sb = P.tile([64, 128], bf16)     # bf16 k of diag chunk j0
            kv_sb = P.tile([128, 32, 128], f8)   # rows 0:64 k^T, 64:128 v^T
            qT_sb = P.tile([64, 4, 512], f8)
            vn_sb = P.tile([128, 32, 80], f8)    # v natural + indicator col (80: DoubleRow lhsT plane step must be %16==0)
            vn0_sb = P.tile([128, 65], bf16)     # bf16 vn for phys chunk 4
            pd0_sb = P.tile([128, 1280], bf16)   # position-0 diag P (bf16)
            tri_sb = P.tile([128, 128], bf16)    # triangle: keep iff col>=chan
            id8_sb = P.tile([128, 128], f8)
            idb_sb = P.tile([64, 64], bf16)
            idf_sb = P.tile([128, 128], f32)

            # input DMAs — weights, then the first xkv slice, then the
            # small tensors, then the rest of the xkv stream
            nc.sync.dma_start(out=wt_sb, in_=wt_d[:, :, :])
            for h in range(2):
                nc.sync.dma_start(
                    out=xkv_sb[:, :, h * 512:(h + 1) * 512],
                    in_=xkv0_d[h][:, :, :])
            nc.sync.dma_start(out=sb_sb, in_=sb_d[:, :])
            for h in range(2):
                nc.sync.dma_start(
                    out=xkv_sb[:, :, 1024 + h * 512:1536 + h * 512],
                    in_=xkv_d[0][:, :, h * 512:(h + 1) * 512])
            for g in range(2, 4):
                nc.sync.dma_start(
                    out=xkv_sb[:, :, g * 1024:(g + 1) * 1024],
                    in_=xkv_d[g - 1][:, :, :])
            # constants
            make_identity(nc, id8_sb[:, :])
            make_identity(nc, idb_sb[:, :])
            make_identity(nc, idf_sb[:, :])
            nc.gpsimd.memset(tri_sb[:, :], 1.0)
            nc.gpsimd.affine_select(
                out=tri_sb[:, :], in_=tri_sb[:, :],
                compare_op=mybir.AluOpType.is_ge, fill=0.0,
                base=0, pattern=[[1, 128]], channel_multiplier=-1)
            nc.gpsimd.memset(vn_sb[:, :, :], WS)
            nc.gpsimd.memset(vn0_sb[:, :], WS)

            # indicator for phys chunks 0..3 comes from data (0 on even
            # cores where those chunks are pads, 32 on odd)
            nc.vector.tensor_copy(vn_sb[:, 0:4, 64:65],
                                  ind_sb[:, :].rearrange("p (j o) -> p j o",
                                                         o=1))

            def proj_kv(blk):
                pt = PSC.tile([128, 2, 512], f32, tag="sc",
                              name=f"pjkv{blk}") \
                    .rearrange("p c n -> p (c n)")[:, 0:512]
                for cp in range(4):
                    nc.tensor.matmul(
                        pt, wt_sb[:, 2 * cp:2 * cp + 2, 0:128],
                        xkv_sb[:, 2 * cp:2 * cp + 2,
                               blk * 512:(blk + 1) * 512],
                        start=(cp == 0), stop=(cp == 3), perf_mode=DR)
                nc.vector.tensor_copy(
                    kv_sb[:, 4 * blk:4 * blk + 4, :]
                    .rearrange("p c n -> p (c n)"), pt)

            def proj_q(i):
                pt = PSC.tile([128, 2, 512], f32, tag="sc",
                              name=f"pjq{i}") \
                    .rearrange("p c n -> p (c n)")[:, 0:512]
                c0 = 512 * (2 * i + 1)
                for cp in range(4):
                    nc.tensor.matmul(
                        pt[0:64, :], wt_sb[:, 2 * cp:2 * cp + 2, 128:192],
                        xkv_sb[:, 2 * cp:2 * cp + 2, c0:c0 + 512],
                        start=(cp == 0), stop=(cp == 3), perf_mode=DR)
                nc.vector.tensor_copy(qT_sb[:, i, :], pt[0:64, :])

            def build_vn(g):
                tp = PM.tile([128, 512], f32, tag="mpt")
                # fp8 PE transpose requires output element step 2: write
                # each chunk sparse ([64 elems @ stride 2]), read strided
                tp8 = tp.bitcast(mybir.dt.float8e4)[:, 0:1024] \
                        .rearrange("p (c n two) -> p c n two", c=8, two=2)
                for j in range(8):
                    ch = 8 * g + j
                    nc.tensor.transpose(
                        tp8[:, j, :, 0:1],
                        kv_sb[64:128, ch, :], id8_sb[64:128, 64:128])
                nc.vector.tensor_copy(
                    vn_sb[:, 8 * g:8 * g + 8, 0:64].rearrange(
                        "p c (n o) -> p c n o", o=1),
                    tp8[:, :, :, 0:1])

            def make_pos(i):
                st = {"op": None, "flushed": 0}
                npairs = EMAIN[i] // 2
                st["npv"] = npairs + 2
                d0 = 8 * i + 4
                if i == 0:
                    pdA = pd0_sb[:, 0:896]
                    pdB = pd0_sb[:, 896:1280]
                else:
                    pdA = PP.tile([128, 896], f8, tag="pdA",
                                  name=f"pdA{i}")
                    pdB = PP.tile([128, 384], f8, tag="pdB",
                                  name=f"pdB{i}")

                def pdsel(jj):
                    if jj < 2:
                        return pdA, TOFF[jj]
                    return pdB, TOFF[jj] - 896

                def getop():
                    if st["op"] is None:
                        st["op"] = POP.tile([80, 512], f32, tag="op",
                                            name=f"op{i}")
                    return st["op"]

                def issue_diag(half):
                    sp = PSC.tile([128, 2, 512], f32, tag="sc")
                    spf = sp.rearrange("p c n -> p (c n)")
                    base = 896 * half
                    for j in (0, 1):
                        jj = 2 * half + j
                        w = 512 - 128 * jj
                        off0 = TOFF[jj] - base
                        if i == 0 and jj == 0:
                            nc.tensor.matmul(
                                spf[:, off0:off0 + 128], k0b_sb[:, :],
                                q0b_sb[:, :], start=True, stop=True)
                            nc.tensor.matmul(
                                spf[:, off0 + 128:off0 + 512],
                                kv_sb[0:64, d0, :],
                                qT_sb[:, i, 128:512],
                                start=True, stop=True)
                        else:
                            nc.tensor.matmul(
                                spf[:, off0:off0 + w],
                                kv_sb[0:64, d0 + jj, :],
                                qT_sb[:, i, 128 * jj:512],
                                start=True, stop=True)
                    wtot = 896 - 512 * half
                    pdt = pdA if half == 0 else pdB
                    nc.scalar.activation(pdt[:, 0:wtot], spf[:, 0:wtot],
                                         Exp, scale=ESC)
                    for j in (0, 1):
                        jj = 2 * half + j
                        pdt, toff = pdsel(jj)
                        nc.gpsimd.tensor_mul(
                            pdt[:, toff:toff + 128],
                            pdt[:, toff:toff + 128], tri_sb)

                def pv_diag(half, first, last):
                    op = getop()
                    for j in (0, 1):
                        jj = 2 * half + j
                        w = 512 - 128 * jj
                        if i == 0 and jj == 0:
                            vAP = vn0_sb[:, :]
                            oAP = op[0:65, 128 * jj:512]
                        else:
                            vAP = vn_sb[:, d0 + jj, :]
                            oAP = op[:, 128 * jj:512]
                        pdt, toff = pdsel(jj)
                        nc.tensor.matmul(
                            oAP, vAP, pdt[:, toff:toff + w],
                            start=(first and j == 0),
                            stop=(last and j == 1))

                def issue_main(pc):
                    sp = PSC.tile([128, 2, 512], f32, tag="sc")
                    for h in range(2):
                        nc.tensor.matmul(
                            sp[:, h, :], kv_sb[0:64, 2 * pc + h, :],
                            qT_sb[:, i, :], start=True, stop=True)
                    if pc in FEXP[i]:
                        pb = PPF.tile([128, 2, 512], f8, tag="pbf")
                        ti = PP.tile([128, 2, 512], mybir.dt.int32,
                                     tag="ti")
                        nc.vector.tensor_scalar(
                            out=ti, in0=sp, scalar1=FE_A, scalar2=FE_B,
                            op0=mybir.AluOpType.mult,
                            op1=mybir.AluOpType.add)
                        if _FEXP_GS:
                            nc.gpsimd.tensor_copy(
                                pb, ti.bitcast(mybir.dt.float32))
                        else:
                            nc.vector.tensor_copy(
                                pb, ti.bitcast(mybir.dt.float32))
                    else:
                        pb = PP.tile([128, 2, 512], f8, tag="pb")
                        nc.scalar.activation(pb, sp, Exp, scale=ESC)
                    return pb

                def pv_main(pb, pc, first, last):
                    nc.tensor.matmul(
                        getop(), vn_sb[:, 2 * pc:2 * pc + 2, :], pb,
                        start=first, stop=last, perf_mode=DR)

                def finalize():
                    op = st["op"]
                    ob = FIN.tile([65, 512], f32, tag="ob")
                    nc.vector.tensor_copy(ob, op[0:65, :])
                    rs = FIN.tile([128, 4, 64], f32, tag="rs")
                    tq = PSC.tile([128, 2, 512], f32, tag="sc")
                    tqr = tq.rearrange("p c n -> p (c n)")[:, 0:288] \
                            .rearrange("p (t w) -> p t w", w=72)
                    for t in range(4):
                        nc.tensor.transpose(
                            tqr[:, t, 0:65], ob[:, t * 128:(t + 1) * 128],
                            idf_sb[0:65, 0:65])
                    rc = FIN.tile([128, 4], f32, tag="rc")
                    rcv = rc.rearrange("p (t o) -> p t o", o=1)
                    nc.vector.reciprocal(rcv, tqr[:, :, 64:65])
                    nc.vector.tensor_mul(rs, tqr[:, :, 0:64],
                                         rcv.broadcast_to([128, 4, 64]))
                    nc.sync.dma_start(
                        out=out_d[i * 512:(i + 1) * 512, :]
                        .rearrange("(t p) h -> p t h", p=128), in_=rs)

                if i == 0:
                    items = [("m", 0), ("m", 1), ("d", 0), ("d", 1)]
                else:
                    items = [("d", 0), ("d", 1)] + [("m", pc)
                                                    for pc in range(npairs)]
                return (st, items, issue_diag, issue_main, pv_diag,
                        pv_main, finalize)

            # bf16 projections for position 0 (exact path for rows 0..511):
            # q over its own columns, k for its diag chunks, v for chunk j0
            def _proj_bf(dst, off):
                pt = PM.tile([128, 512], f32, tag="mpt")
                for c in range(8):
                    nc.tensor.matmul(
                        pt[0:64, 0:128], w0_sb[:, c, off:off + 64],
                        x0_sb[:, c, :], start=(c == 0), stop=(c == 7))
                nc.vector.tensor_copy(dst[:, :], pt[0:64, 0:128])

            def _proj_v0c():
                pt2 = PM.tile([128, 512], f32, tag="mpt")
                for c in range(8):
                    nc.tensor.matmul(
                        pt2[0:64, 0:128], w0_sb[:, c, 128:192],
                        x0_sb[:, c, 0:128], start=(c == 0), stop=(c == 7))
                vt0 = FIN.tile([64, 128], bf16, tag="vt0")
                nc.vector.tensor_copy(vt0, pt2[0:64, 0:128])
                tpb = PM.tile([128, 512], f32, tag="mpt")
                tpb16 = tpb.bitcast(mybir.dt.bfloat16)[:, 0:64]
                nc.tensor.transpose(tpb16, vt0, idb_sb[:, :])
                nc.vector.tensor_copy(vn0_sb[:, 0:64], tpb16)

            v0_fillers = [lambda: _proj_bf(q0b_sb, 0),
                          lambda: _proj_bf(k0b_sb, 64),
                          _proj_v0c]

            # continuous cross-position item stream: QK/exp of position g+1
            # issue while position g's PVs and finalize drain, so neither
            # ACT nor PE idles at position boundaries
            pos = [make_pos(i) for i in range(4)]
            pend, held, fillers = [], [], []


            def flush_one():
                i, kind, data = pend.pop(0)
                st, _, _, _, pv_diag, pv_main, finalize = pos[i]
                first = st["flushed"] == 0
                st["flushed"] += 1
                last = st["flushed"] == st["npv"]
                if kind == "d":
                    pv_diag(data, first, last)
                else:
                    pv_main(data[1], data[0], first, last)
                if last:
                    finalize()

            proj_kv(0)
            proj_kv(1)
            proj_q(0)
            build_vn(0)
            fillers += [(0, f) for f in v0_fillers]
            for g in range(4):
                st, items, issue_diag, issue_main, _, _, _ = pos[g]
                # position boundary: previously-held fast-exp PVs flush
                # before the new position's PVs (keeps op-pool order sane),
                # and any leftover fillers this position depends on run now
                pend.extend(h[1] for h in held)
                held.clear()
                while fillers and fillers[0][0] <= g:
                    fillers.pop(0)[1]()
                if g < 3:
                    fillers += [(g + 1, lambda h=g + 1: proj_kv(2 * h)),
                                (g + 1, lambda h=g + 1: proj_kv(2 * h + 1)),
                                (g + 1, lambda h=g + 1: proj_q(h)),
                                (g + 1, lambda h=g + 1: build_vn(h))]
                for it in items:
                    if it[0] == "d":
                        issue_diag(it[1])
                        pend.append((g, "d", it[1]))
                    else:
                        pb = issue_main(it[1])
                        if it[1] in FEXP[g]:
                            held.append([_FEXP_LAG, (g, "m", (it[1], pb))])
                        else:
                            pend.append((g, "m", (it[1], pb)))
                    for h in held:
                        h[0] -= 1
                    while held and held[0][0] <= 0:
                        pend.append(held.pop(0)[1])
                    if fillers:
                        fillers.pop(0)[1]()
                    if len(pend) > 2:
                        flush_one()
            pend.extend(h[1] for h in held)
            held.clear()
            while pend:
                flush_one()
            for _, f in fillers:
                f()
    nc.compile()
    return nc


def _get_program():
    if "nc" not in _CACHE:
        _CACHE["nc"] = _build_program()
    return _CACHE["nc"]


def _arr(a):
    """[C, N] -> [128, 8, N]: partition-major layout for fast DMA."""
    return np.ascontiguousarray(
        a.reshape(8, 128, a.shape[1]).transpose(1, 0, 2))


def _host_prep(x, Wk, Wq, Wv):
    wt = _arr(np.concatenate([Wk.T * WS, Wv.T * WS, Wq.T * WS],
                             axis=1).astype(F8))
    w0 = _arr(np.concatenate([Wq.T * WS, Wk.T * WS, Wv.T * WS],
                             axis=1).astype(BF16))

    in_maps = []
    for core in range(NCORES):
        b, p = core // 2, core % 2
        xT = x[b].T  # [C, T] f32
        if p == 0:
            xkv = np.concatenate(
                [np.zeros((C, 512), np.float32), xT[:, 0:3584]],
                axis=1).astype(F8)
            x0 = xT[:, 0:128].astype(BF16)
            ind = np.zeros((128, 4), np.float32)
        else:
            xkv = xT.astype(F8)
            x0 = xT[:, 512:640].astype(BF16)
            ind = np.full((128, 4), WS, np.float32)
        m = {f"xkv{g}": _arr(xkv[:, g * 1024:(g + 1) * 1024])
             for g in range(1, 4)}
        m["xkv00"] = _arr(xkv[:, 0:512])
        m["xkv01"] = _arr(xkv[:, 512:1024])
        b = np.empty((128, 5136), np.uint8)
        b[:, 0:3072] = w0.reshape(128, -1).view(np.uint8)
        b[:, 3072:5120] = _arr(x0).reshape(128, -1).view(np.uint8)
        b[:, 5120:5136] = ind.view(np.uint8)
        m.update({"wt": wt, "smallblob": b})
        in_maps.append(m)
    return in_maps


def _gather(results):
    out = np.zeros((B, T, H), dtype=np.float32)
    for core in range(NCORES):
        b, p = core // 2, core % 2
        shard = np.asarray(results[core]["out"], dtype=np.float32)
        for i in range(4):
            g = 2 * i + p
            out[b, 512 * g:512 * g + 512, :] = shard[512 * i:512 * i + 512, :]
    return out


def run(x, Wk, Wq, Wv, trace=False):
    from concourse.bass_utils import run_bass_kernel_spmd

    nc = _get_program()
    in_maps = _host_prep(x, Wk, Wq, Wv)
    res = run_bass_kernel_spmd(
        nc, in_maps, list(range(NCORES)), trace=trace)
    return _gather(res.results), res


def kernel(x, Wk, Wq, Wv):
    out, _ = run(np.asarray(x, dtype=np.float32),
                 np.asarray(Wk, dtype=np.float32),
                 np.asarray(Wq, dtype=np.float32),
                 np.asarray(Wv, dtype=np.float32))
    return out
